# revision 24
# baseline (speedup 1.0000x reference)
"""Trainium2 Bass kernel for nn_AffineAdapter (Gaussian blur + affine grid_sample).

The reference pipeline (separable 8-tap Gaussian blur -> bilinear grid_sample on a
25x25 grid, align_corners=True, zero padding) is linear in x and separable per
axis, so each (b, c) image reduces to   out = Ay @ X @ Ax^T   with Ay, Ax of
shape (25, 512) combining blur taps and bilinear weights.  Output sample row p
only reads the 9 input rows [y0(p)-3, y0(p)+6) ("band") and only a ~362-column
window is ever touched, so just 25*9 = 225 rows x ~362 cols of each 512x512
image carry information.

Distribution/layout: pure data parallel over B*C = 128 images -> 16 images per
core on 8 NeuronCores.  While sharding, the host packs each image's 225 banded
rows (cropped to the exact column span) into a dense block, splitting values
into bf16 hi + lo halves (x = hi + lo exactly; products recover full fp32
accuracy while TensorE runs at bf16 speed with fast weight loads).  The packed
layout is partition-major so each (image, partition) is one contiguous ~3KB run,
read with one 128-partition DMA per image on alternating HWDGE rings.

Per core on-device:
  stage 1:  tmpT[w, p] (per img) = sum_k Xg[k, w] * Ayg[k, p] over the 225
            gathered rows (2 chunks x hi/lo passes; X is the stationary
            operand so the surviving w axis lands on PSUM partitions)
  stage 2:  out[q, (img, p)] = sum_w Ax[q, w] * tmpT[w, (img, p)]  (one fp32
            matmul per w-chunk per image half)
Host computes Ay/Ax from the runtime log_sigma/log_scale inputs and transposes
the gathered (25, 16, 25) per-core outputs back to (B, C, 25, 25).
"""

import sys

if "/opt/trn_rl_repo" not in sys.path:
    sys.path.insert(0, "/opt/trn_rl_repo")

import numpy as np

GRID = 25
K = 7
KH = K // 2          # conv padding = 3
NTAPS = K + 1        # 8 taps (torch arange quirk)
BAND = NTAPS + 1     # 9 rows per output sample row
NG = GRID * BAND     # 225 gathered rows per image
NGP = 256            # padded to 2 x 128 partitions (rows 225.. are zero)
H = W = 512
B, C = 16, 8
N_CORES = 8
NIMG = (B * C) // N_CORES  # images per core
IGRP = 1                   # images packed per DMA


def _softplus(v):
    v = np.asarray(v)
    return np.log1p(np.exp(-np.abs(v))) + np.maximum(v, 0.0)


def _axis_weights(lin, g, scale_ax, n_in):
    """(GRID, n_in) float64 weight matrix + per-sample band starts r0 such that
    the support of row p lies in [r0[p], r0[p] + BAND)."""
    nb = n_in - 1  # blurred length (conv with K+1 taps, pad K//2 shrinks by 1)
    coord = ((lin * np.float32(scale_ax) + np.float32(1.0))
             * np.float32(0.5) * np.float32(nb - 1)).astype(np.float32)
    c0 = np.floor(coord)
    w1 = (coord - c0).astype(np.float64)
    w0 = 1.0 - w1
    A = np.zeros((GRID, n_in), np.float64)
    g64 = g.astype(np.float64)
    r0 = np.zeros(GRID, np.int64)
    for p in range(GRID):
        r0[p] = int(min(max(c0[p] - KH, 0), n_in - BAND))
        for a, wgt in ((0, w0[p]), (1, w1[p])):
            cc = float(c0[p]) + a
            if not (0.0 <= cc <= nb - 1):
                continue  # zero padding_mode: out-of-range corner contributes 0
            ci = int(min(max(cc, 0.0), nb - 1))
            # blurred[ci] = sum_i g[i] * x[ci + i - KH]
            for i in range(NTAPS):
                src = ci + i - KH
                if 0 <= src < n_in:
                    A[p, src] += wgt * g64[i]
    return A, r0


def _build_weights(log_sigma, log_scale):
    # scalar chain in fp32 to mirror the reference
    scale = _softplus(np.asarray(log_scale, np.float32)).astype(np.float32)
    s_min = np.float32(scale.min())
    sigma_min = np.float32(0.0) if s_min >= 1.0 else np.float32(0.44) * (
        np.float32(1.0) / s_min - np.float32(1.0))
    sigma = np.float32(np.sqrt(sigma_min ** 2
                               + _softplus(np.asarray(log_sigma, np.float32)) ** 2))
    taps = np.arange(-(KH + 1), KH + 1, dtype=np.float32)
    g = np.exp(-0.5 * (taps / sigma) ** 2)
    g = g / g.sum()

    lin = np.linspace(-1.0, 1.0, GRID).astype(np.float32)
    Ay, ry = _axis_weights(lin, g, scale[1], H)  # rows scaled by scale[1] (y)
    Ax, _ = _axis_weights(lin, g, scale[0], W)   # cols scaled by scale[0] (x)
    return Ay, Ax, ry


def _col_window(Amat, n_in):
    """[start, start + wlen) covering A's nonzero columns (wlen <= n_in)."""
    used = np.nonzero(Amat.any(axis=0))[0]
    if len(used) == 0:
        return 0, 1
    lo, hi = int(used[0]), int(used[-1]) + 1
    return lo, hi - lo


def _bf16_split(a32):
    import ml_dtypes
    hi = a32.astype(ml_dtypes.bfloat16)
    lo = (a32 - hi.astype(np.float32)).astype(ml_dtypes.bfloat16)
    return hi, lo


_PROGRAM_CACHE = {}


def _build_program(wlen):
    import concourse.tile as tile
    from concourse import bacc, mybir

    f32 = mybir.dt.float32
    bf16 = mybir.dt.bfloat16
    ncw = -(-wlen // 128)
    ms = [min(128, wlen - cw * 128) for cw in range(ncw)]  # stage-1 M per chunk

    nc = bacc.Bacc("TRN2", target_bir_lowering=False, debug=False,
                   num_devices=N_CORES)
    # packed gathered rows: (img, 128 partitions, 2 chunks, hi/lo, window
    # cols) -> per (img, partition) one contiguous 2*2*wlen*2B run
    xs = nc.dram_tensor("xs", [NIMG // IGRP, 128, IGRP, 2, 2, wlen], bf16,
                        kind="ExternalInput")
    # stage-1 rhs per row chunk; cols interleaved (p, t): 2p = hi, 2p+1 = lo
    ayt = nc.dram_tensor("ayt", [2, 128, 2 * GRID], bf16, kind="ExternalInput")
    axt = nc.dram_tensor("axt", [ncw, 128, GRID], f32, kind="ExternalInput")
    out = nc.dram_tensor("out", [GRID, NIMG, GRID], f32, kind="ExternalOutput")

    with tile.TileContext(nc) as tc:
        with (
            tc.tile_pool(name="const", bufs=1) as const_pool,
            tc.tile_pool(name="xp", bufs=min(8, NIMG // IGRP)) as xpool,
            tc.tile_pool(name="ps1", bufs=4, space="PSUM") as psum1,
            tc.tile_pool(name="ps2", bufs=2, space="PSUM") as psum2,
        ):
            aytile = const_pool.tile([128, 2, 2 * GRID], bf16)
            nc.sync.dma_start(out=aytile[:],
                              in_=ayt.rearrange("c p n -> p c n"))
            axtile = const_pool.tile([128, ncw, GRID], f32)
            nc.scalar.dma_start(out=axtile[:],
                                in_=axt.rearrange("c p n -> p c n"))

            tm = const_pool.tile([128, ncw, NIMG, GRID], f32)
            for grp in range(NIMG // IGRP):
                # one full-width DMA per image, ~3KB per partition line;
                # alternate the two HWDGE rings so issue isn't sequencer-bound
                xt = xpool.tile([128, IGRP, 2, 2, wlen], bf16)
                eng = nc.sync if grp % 2 == 0 else nc.scalar
                eng.dma_start(out=xt[:], in_=xs[grp])

                for i4 in range(IGRP):
                    img = grp * IGRP + i4
                    for cw in range(ncw):
                        m = ms[cw]
                        # psum cols interleaved (p, t); single X-axis reduce
                        ps = psum1.tile([128, GRID, 2], f32)
                        for c in range(2):
                            for t in range(2):
                                nc.tensor.matmul(
                                    ps[:m],
                                    xt[:, i4, c, t,
                                       cw * 128:cw * 128 + m],
                                    aytile[:, c, :],
                                    start=(c == 0 and t == 0),
                                    stop=(c == 1 and t == 1),
                                )
                        nc.vector.tensor_reduce(
                            tm[:m, cw, img, :], ps[:m],
                            axis=mybir.AxisListType.X, op=mybir.AluOpType.add)

            # stage 2: per image-half, one fp32 matmul per w-chunk (lets the
            # first half start + stream out while the second half still runs)
            outst = const_pool.tile([GRID, NIMG, GRID], f32)
            HALF = NIMG // 2
            for h in range(2):
                sl = slice(h * HALF, (h + 1) * HALF)
                po = psum2.tile([GRID, HALF, GRID], f32)
                for cw in range(ncw):
                    m = ms[cw]
                    nc.tensor.matmul(
                        po[:],
                        axtile[:m, cw, :],                 # lhsT (K=w, M=q)
                        tm[:m, cw, sl, :],                 # rhs  (K=w, N=(img,p))
                        start=(cw == 0),
                        stop=(cw == ncw - 1),
                    )
                nc.vector.tensor_copy(outst[:, sl, :], po[:])
                eng = nc.sync if h == 0 else nc.scalar
                eng.dma_start(out=out[:, sl, :], in_=outst[:, sl, :])

    nc.compile()
    return nc


def _get_program(wlen):
    if wlen not in _PROGRAM_CACHE:
        _PROGRAM_CACHE[wlen] = _build_program(wlen)
    return _PROGRAM_CACHE[wlen]


def _gather_ay(Ay, ry):
    """Stage-1 rhs chunks: gathered row k = 9*p + j holds Ay[p, ry[p]+j],
    masked so it only feeds output sample p; cols interleaved (p, hi/lo)."""
    ayt64 = np.zeros((2, 128, GRID), np.float64)
    for p in range(GRID):
        for j in range(BAND):
            k = BAND * p + j
            ayt64[k // 128, k % 128, p] = Ay[p, int(ry[p]) + j]
    for p in range(GRID):
        sup = np.nonzero(Ay[p])[0]
        if len(sup) and not (ry[p] <= sup[0] and sup[-1] < ry[p] + BAND):
            raise AssertionError("band does not cover sample support")
    hi, lo = _bf16_split(ayt64.astype(np.float32))
    outw = np.zeros((2, 128, 2 * GRID), hi.dtype)
    outw[:, :, 0::2] = hi
    outw[:, :, 1::2] = lo
    return outw


def _prepare(log_sigma, log_scale):
    Ay, Ax, ry = _build_weights(log_sigma, log_scale)
    w0, wlen = _col_window(Ax, W)
    ncw = -(-wlen // 128)
    ayt = _gather_ay(Ay, ry)

    # axt rows beyond the window span are zero (Ax has no support there)
    pad = np.zeros((GRID, max(0, w0 + ncw * 128 - Ax.shape[1])))
    Aw = np.concatenate([Ax, pad], axis=1)[:, w0:w0 + ncw * 128]
    axt = np.ascontiguousarray(Aw.T.reshape(ncw, 128, GRID)).astype(np.float32)
    return ayt, axt, ry, w0, wlen


def _pack_x(x, ry, w0, wlen):
    """Gather banded rows, crop columns to the exact span, split bf16 hi/lo,
    pad rows to 256.  Returns (B*C/IGRP, 128, IGRP, 2, 2, wlen) bf16."""
    xf = x.reshape(B * C, H, W)
    rows = (np.repeat(np.asarray(ry, np.int64), BAND)
            + np.tile(np.arange(BAND), GRID))        # (225,)
    crop = np.zeros((B * C, NGP, wlen), np.float32)
    crop[:, :NG, :] = xf[:, rows, w0:w0 + wlen]
    hi, lo = _bf16_split(crop)
    xp = np.stack([hi, lo], axis=2)                  # (BC, 256, 2, wlen)
    xp = xp.reshape(B * C, 2, 128, 2, wlen)          # (BC, chunk, part, t, w)
    xp = xp.transpose(0, 2, 1, 3, 4)                 # (BC, part, chunk, t, w)
    xp = xp.reshape(B * C // IGRP, IGRP, 128, 2, 2, wlen)
    return np.ascontiguousarray(xp.transpose(0, 2, 1, 3, 4, 5))


def kernel(x, log_sigma, log_scale):
    from concourse.bass_utils import run_bass_kernel_spmd

    x = np.ascontiguousarray(np.asarray(x, np.float32))
    assert x.shape == (B, C, H, W), x.shape

    ayt, axt, ry, w0, wlen = _prepare(log_sigma, log_scale)
    nc = _get_program(wlen)
    xp = _pack_x(x, ry, w0, wlen)

    nsh = xp.shape[0] // N_CORES
    in_maps = [
        {"xs": xp[i * nsh:(i + 1) * nsh], "ayt": ayt, "axt": axt}
        for i in range(N_CORES)
    ]
    res = run_bass_kernel_spmd(nc, in_maps, core_ids=list(range(N_CORES)))

    out = np.empty((B * C, GRID, GRID), np.float32)
    for i in range(N_CORES):
        # per-core output is (GRID, NIMG, GRID) = (q, img, p)
        out[i * NIMG:(i + 1) * NIMG] = res.results[i]["out"].transpose(1, 2, 0)
    return out.reshape(B, C, GRID, GRID)


# revision 26
# speedup vs baseline: 1.0965x; 1.0965x over previous
"""Trainium2 Bass kernel for nn_AffineAdapter (Gaussian blur + affine grid_sample).

The reference pipeline (separable 8-tap Gaussian blur -> bilinear grid_sample on a
25x25 grid, align_corners=True, zero padding) is linear in x and separable per
axis, so each (b, c) image reduces to   out = Ay @ X @ Ax^T   with Ay, Ax of
shape (25, 512) combining blur taps and bilinear weights.  Output sample row p
only reads the 9 input rows [y0(p)-3, y0(p)+6) ("band") and only a ~362-column
window is ever touched, so just 25*9 = 225 rows x ~362 cols of each 512x512
image carry information.

Distribution/layout: pure data parallel over B*C = 128 images -> 16 images per
core on 8 NeuronCores.  While sharding, the host packs each image's 225 banded
rows (cropped to the exact column span) into a dense block, splitting values
into bf16 hi + lo halves (x = hi + lo exactly; products recover full fp32
accuracy while TensorE runs at bf16 speed with fast weight loads).  The packed
layout is partition-major so each (image, partition) is one contiguous ~3KB run,
read with one 128-partition DMA per image on alternating HWDGE rings.

Per core on-device:
  stage 1:  tmpT[w, p] (per img) = sum_k Xg[k, w] * Ayg[k, p] over the 225
            gathered rows (2 chunks x hi/lo passes; X is the stationary
            operand so the surviving w axis lands on PSUM partitions)
  stage 2:  out[q, (img, p)] = sum_w Ax[q, w] * tmpT[w, (img, p)]  (one fp32
            matmul per w-chunk per image half)
Host computes Ay/Ax from the runtime log_sigma/log_scale inputs and transposes
the gathered (25, 16, 25) per-core outputs back to (B, C, 25, 25).
"""

import sys

if "/opt/trn_rl_repo" not in sys.path:
    sys.path.insert(0, "/opt/trn_rl_repo")

import numpy as np

GRID = 25
K = 7
KH = K // 2          # conv padding = 3
NTAPS = K + 1        # 8 taps (torch arange quirk)
BAND = NTAPS + 1     # 9 rows per output sample row
NG = GRID * BAND     # 225 gathered rows per image
NGP = 256            # padded to 2 x 128 partitions (rows 225.. are zero)
H = W = 512
B, C = 16, 8
N_CORES = 8
NIMG = (B * C) // N_CORES  # images per core
IGRP = 1                   # images packed per DMA


def _softplus(v):
    v = np.asarray(v)
    return np.log1p(np.exp(-np.abs(v))) + np.maximum(v, 0.0)


def _axis_weights(lin, g, scale_ax, n_in):
    """(GRID, n_in) float64 weight matrix + per-sample band starts r0 such that
    the support of row p lies in [r0[p], r0[p] + BAND)."""
    nb = n_in - 1  # blurred length (conv with K+1 taps, pad K//2 shrinks by 1)
    coord = ((lin * np.float32(scale_ax) + np.float32(1.0))
             * np.float32(0.5) * np.float32(nb - 1)).astype(np.float32)
    c0 = np.floor(coord)
    w1 = (coord - c0).astype(np.float64)
    w0 = 1.0 - w1
    A = np.zeros((GRID, n_in), np.float64)
    g64 = g.astype(np.float64)
    r0 = np.zeros(GRID, np.int64)
    for p in range(GRID):
        r0[p] = int(min(max(c0[p] - KH, 0), n_in - BAND))
        for a, wgt in ((0, w0[p]), (1, w1[p])):
            cc = float(c0[p]) + a
            if not (0.0 <= cc <= nb - 1):
                continue  # zero padding_mode: out-of-range corner contributes 0
            ci = int(min(max(cc, 0.0), nb - 1))
            # blurred[ci] = sum_i g[i] * x[ci + i - KH]
            for i in range(NTAPS):
                src = ci + i - KH
                if 0 <= src < n_in:
                    A[p, src] += wgt * g64[i]
    return A, r0


def _build_weights(log_sigma, log_scale):
    # scalar chain in fp32 to mirror the reference
    scale = _softplus(np.asarray(log_scale, np.float32)).astype(np.float32)
    s_min = np.float32(scale.min())
    sigma_min = np.float32(0.0) if s_min >= 1.0 else np.float32(0.44) * (
        np.float32(1.0) / s_min - np.float32(1.0))
    sigma = np.float32(np.sqrt(sigma_min ** 2
                               + _softplus(np.asarray(log_sigma, np.float32)) ** 2))
    taps = np.arange(-(KH + 1), KH + 1, dtype=np.float32)
    g = np.exp(-0.5 * (taps / sigma) ** 2)
    g = g / g.sum()

    lin = np.linspace(-1.0, 1.0, GRID).astype(np.float32)
    Ay, ry = _axis_weights(lin, g, scale[1], H)  # rows scaled by scale[1] (y)
    Ax, _ = _axis_weights(lin, g, scale[0], W)   # cols scaled by scale[0] (x)
    return Ay, Ax, ry


def _col_window(Amat, n_in):
    """[start, start + wlen) covering A's nonzero columns (wlen <= n_in)."""
    used = np.nonzero(Amat.any(axis=0))[0]
    if len(used) == 0:
        return 0, 1
    lo, hi = int(used[0]), int(used[-1]) + 1
    return lo, hi - lo


def _bf16_split(a32):
    import ml_dtypes
    hi = a32.astype(ml_dtypes.bfloat16)
    lo = (a32 - hi.astype(np.float32)).astype(ml_dtypes.bfloat16)
    return hi, lo


_PROGRAM_CACHE = {}


def _build_program(wlen):
    import concourse.tile as tile
    from concourse import bacc, mybir

    f32 = mybir.dt.float32
    bf16 = mybir.dt.bfloat16
    ncw = -(-wlen // 128)
    ms = [min(128, wlen - cw * 128) for cw in range(ncw)]  # stage-1 M per chunk

    nc = bacc.Bacc("TRN2", target_bir_lowering=False, debug=False,
                   num_devices=N_CORES)
    # packed gathered rows: (img, 128 partitions, 2 chunks, hi/lo, window
    # cols) -> per (img, partition) one contiguous 2*2*wlen*2B run
    xs = nc.dram_tensor("xs", [NIMG // IGRP, 128, IGRP, 2, 2, wlen], bf16,
                        kind="ExternalInput")
    # stage-1 rhs per row chunk; cols interleaved (p, t): 2p = hi, 2p+1 = lo
    ayt = nc.dram_tensor("ayt", [2, 128, 2 * GRID], bf16, kind="ExternalInput")
    axt = nc.dram_tensor("axt", [ncw, 128, GRID], f32, kind="ExternalInput")
    out = nc.dram_tensor("out", [GRID, NIMG, GRID], f32, kind="ExternalOutput")

    with tile.TileContext(nc) as tc:
        with (
            tc.tile_pool(name="const", bufs=1) as const_pool,
            tc.tile_pool(name="xp", bufs=min(8, NIMG // IGRP)) as xpool,
            tc.tile_pool(name="ps1", bufs=4, space="PSUM") as psum1,
            tc.tile_pool(name="ps2", bufs=2, space="PSUM") as psum2,
        ):
            aytile = const_pool.tile([128, 2, 2 * GRID], bf16)
            nc.sync.dma_start(out=aytile[:],
                              in_=ayt.rearrange("c p n -> p c n"))
            axtile = const_pool.tile([128, ncw, GRID], f32)
            nc.scalar.dma_start(out=axtile[:],
                                in_=axt.rearrange("c p n -> p c n"))

            tm = const_pool.tile([128, ncw, NIMG, GRID], f32)
            for grp in range(NIMG // IGRP):
                # one full-width DMA per image, ~3KB per partition line;
                # alternate the two HWDGE rings so issue isn't sequencer-bound
                xt = xpool.tile([128, IGRP, 2, 2, wlen], bf16)
                eng = nc.sync if grp % 2 == 0 else nc.scalar
                eng.dma_start(out=xt[:], in_=xs[grp])

                for i4 in range(IGRP):
                    img = grp * IGRP + i4
                    for cw in range(ncw):
                        m = ms[cw]
                        # psum cols interleaved (p, t); single X-axis reduce
                        ps = psum1.tile([128, GRID, 2], f32)
                        for c in range(2):
                            for t in range(2):
                                nc.tensor.matmul(
                                    ps[:m],
                                    xt[:, i4, c, t,
                                       cw * 128:cw * 128 + m],
                                    aytile[:, c, :],
                                    start=(c == 0 and t == 0),
                                    stop=(c == 1 and t == 1),
                                )
                        nc.vector.tensor_reduce(
                            tm[:m, cw, img, :], ps[:m],
                            axis=mybir.AxisListType.X, op=mybir.AluOpType.add)

            # stage 2: per image-half, one fp32 matmul per w-chunk (lets the
            # first half start + stream out while the second half still runs)
            outst = const_pool.tile([GRID, NIMG, GRID], f32)
            HALF = NIMG // 2
            for h in range(2):
                sl = slice(h * HALF, (h + 1) * HALF)
                po = psum2.tile([GRID, HALF, GRID], f32)
                for cw in range(ncw):
                    m = ms[cw]
                    nc.tensor.matmul(
                        po[:],
                        axtile[:m, cw, :],                 # lhsT (K=w, M=q)
                        tm[:m, cw, sl, :],                 # rhs  (K=w, N=(img,p))
                        start=(cw == 0),
                        stop=(cw == ncw - 1),
                    )
                nc.vector.tensor_copy(outst[:, sl, :], po[:])
                eng = nc.sync if h == 0 else nc.scalar
                eng.dma_start(out=out[:, sl, :], in_=outst[:, sl, :])

    nc.compile()
    return nc


def _get_program(wlen):
    if wlen not in _PROGRAM_CACHE:
        _PROGRAM_CACHE[wlen] = _build_program(wlen)
    return _PROGRAM_CACHE[wlen]


def _gather_ay(Ay, ry):
    """Stage-1 rhs chunks: gathered row k = 9*p + j holds Ay[p, ry[p]+j],
    masked so it only feeds output sample p; cols interleaved (p, hi/lo)."""
    ayt64 = np.zeros((2, 128, GRID), np.float64)
    for p in range(GRID):
        for j in range(BAND):
            k = BAND * p + j
            ayt64[k // 128, k % 128, p] = Ay[p, int(ry[p]) + j]
    for p in range(GRID):
        sup = np.nonzero(Ay[p])[0]
        if len(sup) and not (ry[p] <= sup[0] and sup[-1] < ry[p] + BAND):
            raise AssertionError("band does not cover sample support")
    hi, lo = _bf16_split(ayt64.astype(np.float32))
    outw = np.zeros((2, 128, 2 * GRID), hi.dtype)
    outw[:, :, 0::2] = hi
    outw[:, :, 1::2] = lo
    return outw


def _prepare(log_sigma, log_scale):
    Ay, Ax, ry = _build_weights(log_sigma, log_scale)
    w0, wlen = _col_window(Ax, W)
    ncw = -(-wlen // 128)
    ayt = _gather_ay(Ay, ry)

    # axt rows beyond the window span are zero (Ax has no support there)
    pad = np.zeros((GRID, max(0, w0 + ncw * 128 - Ax.shape[1])))
    Aw = np.concatenate([Ax, pad], axis=1)[:, w0:w0 + ncw * 128]
    axt = np.ascontiguousarray(Aw.T.reshape(ncw, 128, GRID)).astype(np.float32)
    return ayt, axt, ry, w0, wlen


def _pack_x(x, ry, w0, wlen):
    """Gather banded rows, crop columns to the exact span, split bf16 hi/lo,
    pad rows to 256.  Returns (B*C/IGRP, 128, IGRP, 2, 2, wlen) bf16."""
    xf = x.reshape(B * C, H, W)
    rows = (np.repeat(np.asarray(ry, np.int64), BAND)
            + np.tile(np.arange(BAND), GRID))        # (225,)
    crop = np.zeros((B * C, NGP, wlen), np.float32)
    crop[:, :NG, :] = xf[:, rows, w0:w0 + wlen]
    hi, lo = _bf16_split(crop)
    xp = np.stack([hi, lo], axis=2)                  # (BC, 256, 2, wlen)
    xp = xp.reshape(B * C, 2, 128, 2, wlen)          # (BC, chunk, part, t, w)
    xp = xp.transpose(0, 2, 1, 3, 4)                 # (BC, part, chunk, t, w)
    xp = xp.reshape(B * C // IGRP, IGRP, 128, 2, 2, wlen)
    return np.ascontiguousarray(xp.transpose(0, 2, 1, 3, 4, 5))


def kernel(x, log_sigma, log_scale):
    from concourse.bass_utils import run_bass_kernel_spmd

    x = np.ascontiguousarray(np.asarray(x, np.float32))
    assert x.shape == (B, C, H, W), x.shape

    ayt, axt, ry, w0, wlen = _prepare(log_sigma, log_scale)
    nc = _get_program(wlen)
    xp = _pack_x(x, ry, w0, wlen)

    nsh = xp.shape[0] // N_CORES
    in_maps = [
        {"xs": xp[i * nsh:(i + 1) * nsh], "ayt": ayt, "axt": axt}
        for i in range(N_CORES)
    ]
    res = run_bass_kernel_spmd(nc, in_maps, core_ids=list(range(N_CORES)))

    out = np.empty((B * C, GRID, GRID), np.float32)
    for i in range(N_CORES):
        # per-core output is (GRID, NIMG, GRID) = (q, img, p)
        out[i * NIMG:(i + 1) * NIMG] = res.results[i]["out"].transpose(1, 2, 0)
    return out.reshape(B, C, GRID, GRID)


# revision 27
# speedup vs baseline: 1.1312x; 1.0317x over previous
"""Trainium2 Bass kernel for nn_AffineAdapter (Gaussian blur + affine grid_sample).

The reference pipeline (separable 8-tap Gaussian blur -> bilinear grid_sample on a
25x25 grid, align_corners=True, zero padding) is linear in x and separable per
axis, so each (b, c) image reduces to   out = Ay @ X @ Ax^T   with Ay, Ax of
shape (25, 512) combining blur taps and bilinear weights.  Output sample row p
only reads the 9 input rows [y0(p)-3, y0(p)+6) ("band") and only a ~362-column
window is ever touched, so just 25*9 = 225 rows x ~362 cols of each 512x512
image carry information.

Distribution/layout: pure data parallel over B*C = 128 images -> 16 images per
core on 8 NeuronCores.  While sharding, the host packs each image's 225 banded
rows (cropped to the exact column span) into a dense block, splitting values
into bf16 hi + lo halves (x = hi + lo exactly; products recover full fp32
accuracy while TensorE runs at bf16 speed with fast weight loads).  The packed
layout is partition-major so each (image, partition) is one contiguous ~3KB run,
read with one 128-partition DMA per image on alternating HWDGE rings.

Per core on-device:
  stage 1:  tmpT[w, p] (per img) = sum_k Xg[k, w] * Ayg[k, p] over the 225
            gathered rows (2 chunks x hi/lo passes; X is the stationary
            operand so the surviving w axis lands on PSUM partitions)
  stage 2:  out[q, (img, p)] = sum_w Ax[q, w] * tmpT[w, (img, p)]  (one fp32
            matmul per w-chunk per image half)
Host computes Ay/Ax from the runtime log_sigma/log_scale inputs and transposes
the gathered (25, 16, 25) per-core outputs back to (B, C, 25, 25).
"""

import sys

if "/opt/trn_rl_repo" not in sys.path:
    sys.path.insert(0, "/opt/trn_rl_repo")

import numpy as np

GRID = 25
K = 7
KH = K // 2          # conv padding = 3
NTAPS = K + 1        # 8 taps (torch arange quirk)
BAND = NTAPS + 1     # 9 rows per output sample row
NG = GRID * BAND     # 225 gathered rows per image
NGP = 256            # padded to 2 x 128 partitions (rows 225.. are zero)
H = W = 512
B, C = 16, 8
N_CORES = 8
NIMG = (B * C) // N_CORES  # images per core
IGRP = 1                   # images packed per DMA


def _softplus(v):
    v = np.asarray(v)
    return np.log1p(np.exp(-np.abs(v))) + np.maximum(v, 0.0)


def _axis_weights(lin, g, scale_ax, n_in):
    """(GRID, n_in) float64 weight matrix + per-sample band starts r0 such that
    the support of row p lies in [r0[p], r0[p] + BAND)."""
    nb = n_in - 1  # blurred length (conv with K+1 taps, pad K//2 shrinks by 1)
    coord = ((lin * np.float32(scale_ax) + np.float32(1.0))
             * np.float32(0.5) * np.float32(nb - 1)).astype(np.float32)
    c0 = np.floor(coord)
    w1 = (coord - c0).astype(np.float64)
    w0 = 1.0 - w1
    A = np.zeros((GRID, n_in), np.float64)
    g64 = g.astype(np.float64)
    r0 = np.zeros(GRID, np.int64)
    for p in range(GRID):
        r0[p] = int(min(max(c0[p] - KH, 0), n_in - BAND))
        for a, wgt in ((0, w0[p]), (1, w1[p])):
            cc = float(c0[p]) + a
            if not (0.0 <= cc <= nb - 1):
                continue  # zero padding_mode: out-of-range corner contributes 0
            ci = int(min(max(cc, 0.0), nb - 1))
            # blurred[ci] = sum_i g[i] * x[ci + i - KH]
            for i in range(NTAPS):
                src = ci + i - KH
                if 0 <= src < n_in:
                    A[p, src] += wgt * g64[i]
    return A, r0


def _build_weights(log_sigma, log_scale):
    # scalar chain in fp32 to mirror the reference
    scale = _softplus(np.asarray(log_scale, np.float32)).astype(np.float32)
    s_min = np.float32(scale.min())
    sigma_min = np.float32(0.0) if s_min >= 1.0 else np.float32(0.44) * (
        np.float32(1.0) / s_min - np.float32(1.0))
    sigma = np.float32(np.sqrt(sigma_min ** 2
                               + _softplus(np.asarray(log_sigma, np.float32)) ** 2))
    taps = np.arange(-(KH + 1), KH + 1, dtype=np.float32)
    g = np.exp(-0.5 * (taps / sigma) ** 2)
    g = g / g.sum()

    lin = np.linspace(-1.0, 1.0, GRID).astype(np.float32)
    Ay, ry = _axis_weights(lin, g, scale[1], H)  # rows scaled by scale[1] (y)
    Ax, _ = _axis_weights(lin, g, scale[0], W)   # cols scaled by scale[0] (x)
    return Ay, Ax, ry


def _col_window(Amat, n_in):
    """[start, start + wlen) covering A's nonzero columns (wlen <= n_in)."""
    used = np.nonzero(Amat.any(axis=0))[0]
    if len(used) == 0:
        return 0, 1
    lo, hi = int(used[0]), int(used[-1]) + 1
    return lo, hi - lo


def _bf16_split(a32):
    import ml_dtypes
    hi = a32.astype(ml_dtypes.bfloat16)
    lo = (a32 - hi.astype(np.float32)).astype(ml_dtypes.bfloat16)
    return hi, lo


_PROGRAM_CACHE = {}


def _build_program(wlen):
    import concourse.tile as tile
    from concourse import bacc, mybir

    f32 = mybir.dt.float32
    bf16 = mybir.dt.bfloat16
    ncw = -(-wlen // 128)
    ms = [min(128, wlen - cw * 128) for cw in range(ncw)]  # stage-1 M per chunk

    nc = bacc.Bacc("TRN2", target_bir_lowering=False, debug=False,
                   num_devices=N_CORES)
    # packed gathered rows: (img, 128 partitions, 2 chunks, hi/lo, window
    # cols) -> per (img, partition) one contiguous 2*2*wlen*2B run
    xs = nc.dram_tensor("xs", [NIMG // IGRP, 128, IGRP, 2, 2, wlen], bf16,
                        kind="ExternalInput")
    # stage-1 rhs per row chunk; cols interleaved (p, t): 2p = hi, 2p+1 = lo
    ayt = nc.dram_tensor("ayt", [2, 128, 2 * GRID], bf16, kind="ExternalInput")
    axt = nc.dram_tensor("axt", [ncw, 128, GRID], f32, kind="ExternalInput")
    out = nc.dram_tensor("out", [GRID, NIMG, GRID], f32, kind="ExternalOutput")

    with tile.TileContext(nc) as tc:
        with (
            tc.tile_pool(name="const", bufs=1) as const_pool,
            tc.tile_pool(name="xp", bufs=min(8, NIMG // IGRP)) as xpool,
            tc.tile_pool(name="ps1", bufs=4, space="PSUM") as psum1,
            tc.tile_pool(name="ps2", bufs=2, space="PSUM") as psum2,
        ):
            aytile = const_pool.tile([128, 2, 2 * GRID], bf16)
            nc.sync.dma_start(out=aytile[:],
                              in_=ayt.rearrange("c p n -> p c n"))
            axtile = const_pool.tile([128, ncw, GRID], f32)
            nc.scalar.dma_start(out=axtile[:],
                                in_=axt.rearrange("c p n -> p c n"))

            tm = const_pool.tile([128, ncw, NIMG, GRID], f32)
            for grp in range(NIMG // IGRP):
                # one full-width DMA per image, ~3KB per partition line;
                # alternate the two HWDGE rings so issue isn't sequencer-bound
                xt = xpool.tile([128, IGRP, 2, 2, wlen], bf16)
                eng = nc.sync if grp % 2 == 0 else nc.scalar
                eng.dma_start(out=xt[:], in_=xs[grp])

                for i4 in range(IGRP):
                    img = grp * IGRP + i4
                    for cw in range(ncw):
                        m = ms[cw]
                        # psum cols interleaved (p, t); single X-axis reduce
                        ps = psum1.tile([128, GRID, 2], f32)
                        for c in range(2):
                            for t in range(2):
                                nc.tensor.matmul(
                                    ps[:m],
                                    xt[:, i4, c, t,
                                       cw * 128:cw * 128 + m],
                                    aytile[:, c, :],
                                    start=(c == 0 and t == 0),
                                    stop=(c == 1 and t == 1),
                                )
                        nc.vector.tensor_reduce(
                            tm[:m, cw, img, :], ps[:m],
                            axis=mybir.AxisListType.X, op=mybir.AluOpType.add)

            # stage 2: per image-quarter, one fp32 matmul per w-chunk (each
            # quarter streams out while later quarters' stage 1 still runs)
            outst = const_pool.tile([GRID, NIMG, GRID], f32)
            HALF = NIMG // 4
            for h in range(4):
                sl = slice(h * HALF, (h + 1) * HALF)
                po = psum2.tile([GRID, HALF, GRID], f32)
                for cw in range(ncw):
                    m = ms[cw]
                    nc.tensor.matmul(
                        po[:],
                        axtile[:m, cw, :],                 # lhsT (K=w, M=q)
                        tm[:m, cw, sl, :],                 # rhs  (K=w, N=(img,p))
                        start=(cw == 0),
                        stop=(cw == ncw - 1),
                    )
                nc.vector.tensor_copy(outst[:, sl, :], po[:])
                eng = nc.sync if h % 2 == 0 else nc.scalar
                eng.dma_start(out=out[:, sl, :], in_=outst[:, sl, :])

    nc.compile()
    return nc


def _get_program(wlen):
    if wlen not in _PROGRAM_CACHE:
        _PROGRAM_CACHE[wlen] = _build_program(wlen)
    return _PROGRAM_CACHE[wlen]


def _gather_ay(Ay, ry):
    """Stage-1 rhs chunks: gathered row k = 9*p + j holds Ay[p, ry[p]+j],
    masked so it only feeds output sample p; cols interleaved (p, hi/lo)."""
    ayt64 = np.zeros((2, 128, GRID), np.float64)
    for p in range(GRID):
        for j in range(BAND):
            k = BAND * p + j
            ayt64[k // 128, k % 128, p] = Ay[p, int(ry[p]) + j]
    for p in range(GRID):
        sup = np.nonzero(Ay[p])[0]
        if len(sup) and not (ry[p] <= sup[0] and sup[-1] < ry[p] + BAND):
            raise AssertionError("band does not cover sample support")
    hi, lo = _bf16_split(ayt64.astype(np.float32))
    outw = np.zeros((2, 128, 2 * GRID), hi.dtype)
    outw[:, :, 0::2] = hi
    outw[:, :, 1::2] = lo
    return outw


def _prepare(log_sigma, log_scale):
    Ay, Ax, ry = _build_weights(log_sigma, log_scale)
    w0, wlen = _col_window(Ax, W)
    ncw = -(-wlen // 128)
    ayt = _gather_ay(Ay, ry)

    # axt rows beyond the window span are zero (Ax has no support there)
    pad = np.zeros((GRID, max(0, w0 + ncw * 128 - Ax.shape[1])))
    Aw = np.concatenate([Ax, pad], axis=1)[:, w0:w0 + ncw * 128]
    axt = np.ascontiguousarray(Aw.T.reshape(ncw, 128, GRID)).astype(np.float32)
    return ayt, axt, ry, w0, wlen


def _pack_x(x, ry, w0, wlen):
    """Gather banded rows, crop columns to the exact span, split bf16 hi/lo,
    pad rows to 256.  Returns (B*C/IGRP, 128, IGRP, 2, 2, wlen) bf16."""
    xf = x.reshape(B * C, H, W)
    rows = (np.repeat(np.asarray(ry, np.int64), BAND)
            + np.tile(np.arange(BAND), GRID))        # (225,)
    crop = np.zeros((B * C, NGP, wlen), np.float32)
    crop[:, :NG, :] = xf[:, rows, w0:w0 + wlen]
    hi, lo = _bf16_split(crop)
    xp = np.stack([hi, lo], axis=2)                  # (BC, 256, 2, wlen)
    xp = xp.reshape(B * C, 2, 128, 2, wlen)          # (BC, chunk, part, t, w)
    xp = xp.transpose(0, 2, 1, 3, 4)                 # (BC, part, chunk, t, w)
    xp = xp.reshape(B * C // IGRP, IGRP, 128, 2, 2, wlen)
    return np.ascontiguousarray(xp.transpose(0, 2, 1, 3, 4, 5))


def kernel(x, log_sigma, log_scale):
    from concourse.bass_utils import run_bass_kernel_spmd

    x = np.ascontiguousarray(np.asarray(x, np.float32))
    assert x.shape == (B, C, H, W), x.shape

    ayt, axt, ry, w0, wlen = _prepare(log_sigma, log_scale)
    nc = _get_program(wlen)
    xp = _pack_x(x, ry, w0, wlen)

    nsh = xp.shape[0] // N_CORES
    in_maps = [
        {"xs": xp[i * nsh:(i + 1) * nsh], "ayt": ayt, "axt": axt}
        for i in range(N_CORES)
    ]
    res = run_bass_kernel_spmd(nc, in_maps, core_ids=list(range(N_CORES)))

    out = np.empty((B * C, GRID, GRID), np.float32)
    for i in range(N_CORES):
        # per-core output is (GRID, NIMG, GRID) = (q, img, p)
        out[i * NIMG:(i + 1) * NIMG] = res.results[i]["out"].transpose(1, 2, 0)
    return out.reshape(B, C, GRID, GRID)
